# revision 26
# baseline (speedup 1.0000x reference)
"""DampedOscillator Trainium2 kernel.

Math: per mode m, damp/freq are constant in time, so with t = 128*B + u,
  exp(-damp*t/SR)          = E2[m,B] * E1[m,u]
  sin(2*pi*wd*t)           = S2[m,B]*C1[m,u] + C2[m,B]*S1[m,u]
so the amp-weighted mode sum
  signal[a, 128B+u] = sum_m P[a,m,B]*Q[m,u] + R[a,m,B]*W[m,u]
is one K=128 matmul per audio channel (tables host-built, O(M*(128+256))),
producing the signal directly in [u, B] layout.

The grouped long conv out[t] = sum_k g[k]*x[t-k] (g = time-flipped forces):
  out[128*T1 + t0] = sum_d sum_s M_d[s, t0] * x[s, T1-d],
  M_d[s, t0] = gph[s + 128*d + t0]
as 33 accumulating matmuls per channel with the Toeplitz slice M_d stationary
(reused across 256 moving columns) and the signal moving.

All matmuls run in bf16 hi/lo split form (hi*hi + hi*lo + lo*hi), keeping
~16 mantissa bits end to end at 1 PE column/cycle. Toeplitz tiles are built
on-device by overlapping (im2col) DMAs from host-split bf16 force vectors,
chunked along the tap axis and ordered so the PE can start as soon as the
first chunk lands while the rest stream in behind it.

Output lands as [t0, T1] per channel; the host transposes to linear time.

Sharding: data parallel over the 16 audio channels, 2 per NeuronCore.
"""
import ml_dtypes
import numpy as np

import bass_rust
import concourse.bass as bass
import concourse.mybir as mybir
from concourse import bacc
from concourse.bass_utils import run_bass_kernel_spmd

F32 = mybir.dt.float32
BF16 = mybir.dt.bfloat16
BF = ml_dtypes.bfloat16

SR = 16000.0
T = 32768
A = 16
MODES = 64
F = 4096
NB = 256          # number of 128-sample blocks
U = 128           # samples per block
D = 33            # kernel chunks (4096/128 + 1 boundary)
NCORES = 8
APC = A // NCORES  # audio channels per core
GPH = F + 256      # padded g vector length (127 zeros + 4096 + 129 zeros)
DGROUPS = (3, 4, 5, 7, 7, 7)  # d-axis DMA chunking for load/compute overlap

_program_cache = {}


def _toeplitz_src(src_t, a, d0, dn):
    src = src_t[a].copy()
    src.ap = bass_rust.VecI64Pair([[1, 128], [1, dn * 128]])
    src.offset = src.offset + d0 * 128
    return src


def _build_program_raw():
    """Hand-scheduled variant: no TileContext, manual semaphores, minimal
    DMA count (issue costs ~0.65us each), per-d weight-load pairing."""
    if "nc_raw" in _program_cache:
        return _program_cache["nc_raw"]

    nc = bacc.Bacc("TRN2", debug=False)

    qw_in = nc.dram_tensor("qwcat", [128, 2, U], BF16, kind="ExternalInput")
    pr_in = nc.dram_tensor("prcat", [APC, 128, 2, NB], BF16, kind="ExternalInput")
    g_in = nc.dram_tensor("mtab", [APC, 128, 2, D * 128], BF16,
                          kind="ExternalInput")
    out_t = nc.dram_tensor("out", [APC, U, NB], F32, kind="ExternalOutput")

    dg = list(DGROUPS)
    dg_off = [sum(dg[:i]) for i in range(len(dg))]
    import contextlib
    with contextlib.ExitStack() as ctx:
        sb = lambda shape, dt, name: ctx.enter_context(
            nc.sbuf_tensor(name, shape, dt))
        ps = lambda shape, name: ctx.enter_context(
            nc.psum_tensor(name, shape, F32))
        sem = lambda name: ctx.enter_context(nc.semaphore(name))

        qw = sb([128, 2, U], BF16, "qw_t")        # [k, hi/lo, u]
        pr = [sb([128, 2, NB], BF16, f"pr_t{a}") for a in range(APC)]
        mt = [[sb([128, 2, dn * 128], BF16, f"mt{a}g{g}")
               for g, dn in enumerate(dg)] for a in range(APC)]
        xh = [sb([128, 32 + NB], BF16, f"xh_t{a}") for a in range(APC)]
        xl = [sb([128, 32 + NB], BF16, f"xl_t{a}") for a in range(APC)]
        ob = [sb([128, NB], F32, f"ob_t{a}") for a in range(APC)]
        sig = [ps([128, NB], f"sig_t{a}") for a in range(APC)]
        acc = [ps([128, NB], f"acc_t{a}") for a in range(APC)]
        wup_sb = sb([128, 512], BF16, "wup_sb")
        wup_ps = ps([128, 512], "wup_ps")

        s_d0 = sem("s_d0")          # sync smalls: qwcat + prcat0
        s_d1 = sem("s_d1")          # gpsimd smalls: prcat1
        s_m0 = [sem(f"s_m0g{g}") for g in range(len(dg))]
        s_m1 = [sem(f"s_m1g{g}") for g in range(len(dg))]
        s_sig = sem("s_sig")
        s_split = sem("s_split")
        s_acc = sem("s_acc")
        s_ob = sem("s_ob")
        s_outd = sem("s_outd")

        def toe_src(a, d0, dn):
            return g_in[a, :, :, d0 * 128:(d0 + dn) * 128]

        with nc.Block() as block:
            @block.sync
            def _(sync):
                sync.dma_start(qw[:], qw_in[:]).then_inc(s_d0, 16)
                for g in range(1, len(dg), 2):
                    sync.dma_start(mt[0][g][:], toe_src(0, dg_off[g], dg[g])
                                   ).then_inc(s_m0[g], 16)
                for g in range(0, len(dg), 2):
                    sync.dma_start(mt[1][g][:], toe_src(1, dg_off[g], dg[g])
                                   ).then_inc(s_m1[g], 16)

            @block.scalar
            def _(sc):
                sc.dma_start(pr[0][:], pr_in[0]).then_inc(s_d0, 16)
                for g in range(0, len(dg), 2):
                    sc.dma_start(mt[0][g][:], toe_src(0, dg_off[g], dg[g])
                                 ).then_inc(s_m0[g], 16)
                for g in range(1, len(dg), 2):
                    sc.dma_start(mt[1][g][:], toe_src(1, dg_off[g], dg[g])
                                 ).then_inc(s_m1[g], 16)

            @block.gpsimd
            def _(gp):
                gp.dma_start(pr[1][:], pr_in[1]).then_inc(s_d1, 16)
                gp.wait_ge(s_ob, 1)
                gp.dma_start(out_t[0, 0:64], ob[0][0:64, :]).then_inc(s_outd, 16)
                gp.wait_ge(s_ob, 2)
                gp.dma_start(out_t[0, 64:128], ob[0][64:128, :]).then_inc(s_outd, 16)
                gp.wait_ge(s_ob, 3)
                gp.dma_start(out_t[1, 0:64], ob[1][0:64, :]).then_inc(s_outd, 16)
                gp.wait_ge(s_ob, 4)
                gp.dma_start(out_t[1, 64:128], ob[1][64:128, :]).then_inc(s_outd, 16)
                gp.wait_ge(s_outd, 64)

            @block.tensor
            def _(pe):
                # HAM warmup: keep the PE busy on scratch data while input
                # DMAs stream, so the real matmuls run at the warm clock
                for _ in range(13):
                    nc.tensor.matmul(wup_ps[:], wup_sb[:, 0:128], wup_sb[:],
                                     start=True, stop=True)
                pe.wait_ge(s_d0, 32)
                nc.tensor.matmul(sig[0][:], qw[:, 0], pr[0][:, 0], start=True, stop=False)
                nc.tensor.matmul(sig[0][:], qw[:, 0], pr[0][:, 1], start=False, stop=False)
                nc.tensor.matmul(sig[0][:], qw[:, 1], pr[0][:, 0], start=False,
                                 stop=True).then_inc(s_sig, 1)
                pe.wait_ge(s_d1, 16)
                nc.tensor.matmul(sig[1][:], qw[:, 0], pr[1][:, 0], start=True, stop=False)
                nc.tensor.matmul(sig[1][:], qw[:, 0], pr[1][:, 1], start=False, stop=False)
                nc.tensor.matmul(sig[1][:], qw[:, 1], pr[1][:, 0], start=False,
                                 stop=True).then_inc(s_sig, 1)

                def conv(a):
                    k = 0
                    n = 3 * D
                    inst = None
                    for g, dn in enumerate(dg):
                        pe.wait_ge(s_m0[g] if a == 0 else s_m1[g], 16)
                        for d in range(dg_off[g], dg_off[g] + dn):
                            hi = mt[a][g][:, 0, (d - dg_off[g]) * 128:
                                          (d - dg_off[g] + 1) * 128]
                            for rhs in (xh[a], xl[a]):
                                inst = nc.tensor.matmul(
                                    acc[a][:], hi, rhs[:, 32 - d:32 - d + NB],
                                    start=(k == 0), stop=(k == n - 1))
                                k += 1
                    for g, dn in enumerate(dg):
                        for d in range(dg_off[g], dg_off[g] + dn):
                            lo = mt[a][g][:, 1, (d - dg_off[g]) * 128:
                                          (d - dg_off[g] + 1) * 128]
                            inst = nc.tensor.matmul(
                                acc[a][:], lo, xh[a][:, 32 - d:32 - d + NB],
                                start=(k == 0), stop=(k == n - 1))
                            k += 1
                    inst.then_inc(s_acc, 1)

                pe.wait_ge(s_split, 2)
                conv(0)
                pe.wait_ge(s_split, 4)
                conv(1)

            @block.vector
            def _(v):
                for a in range(APC):
                    nc.vector.memset(xh[a][:, 0:32], 0.0)
                    nc.vector.memset(xl[a][:, 0:32], 0.0)
                for a in range(APC):
                    v.wait_ge(s_sig, a + 1)
                    nc.vector.tensor_copy(xh[a][:, 32:], sig[a][:]).then_inc(
                        s_split, 1)
                    nc.vector.tensor_sub(xl[a][:, 32:], sig[a][:],
                                         xh[a][:, 32:]).then_inc(s_split, 1)
                for a in range(APC):
                    v.wait_ge(s_acc, a + 1)
                    nc.vector.tensor_copy(ob[a][0:64, :],
                                          acc[a][0:64, :]).then_inc(s_ob, 1)
                    nc.vector.tensor_copy(ob[a][64:128, :],
                                          acc[a][64:128, :]).then_inc(s_ob, 1)

    nc.compile()
    _program_cache["nc_raw"] = nc
    return nc


def _host_tables(freq_linear, amp_value, alpha_params, beta_params, forces):
    """Derive per-mode constants and the synthesis tables in float64."""
    LOG10 = np.log(10.0)
    amp = 2.0 * (1.0 / (1.0 + np.exp(-amp_value.astype(np.float64)))) ** LOG10 + 1e-7

    def wsum(p, v):
        x = np.log1p(np.exp(p.astype(np.float64)))
        x = x / x.sum(-1, keepdims=True)
        return (v * x).sum(-1)

    alpha_list = np.exp(np.linspace(np.log(0.6), np.log(60.0), 64))
    beta_list = np.exp(np.linspace(np.log(2e-8), np.log(2e-6), 64))
    alpha = wsum(alpha_params, alpha_list)[0, :, 0]
    beta = wsum(beta_params, beta_list)[0, :, 0]
    fl = freq_linear.astype(np.float64)
    lbd = (2.0 * np.pi * fl) ** 2
    damp = 0.5 * (alpha + beta * lbd)
    freq = np.sqrt(lbd - damp ** 2) / (2.0 * np.pi)
    wd = freq / SR

    Bv = np.arange(NB, dtype=np.float64)
    uv = np.arange(U, dtype=np.float64)
    E2 = np.exp(-damp[:, None] * 128.0 * Bv[None, :] / SR)
    ph2 = (wd[:, None] * 128.0 * Bv[None, :]) % 1.0
    S2, C2 = np.sin(2 * np.pi * ph2), np.cos(2 * np.pi * ph2)
    E1 = np.exp(-damp[:, None] * (uv[None, :] + 1.0) / SR)
    ph1 = wd[:, None] * (uv[None, :] + 1.0)
    S1, C1 = np.sin(2 * np.pi * ph1), np.cos(2 * np.pi * ph1)

    Q = E1 * C1                                   # (M, U)
    W = E1 * S1
    P = amp[:, :, 0, None] * E2[None] * S2[None]  # (A, M, NB)
    R = amp[:, :, 0, None] * E2[None] * C2[None]

    # device lhsT: qw[k, s'] with s' = 127-u (reversed sample order), K = [Q; W]
    qw = np.concatenate([Q[:, ::-1], W[:, ::-1]], axis=0).astype(np.float32)  # (128, U)
    pr = np.concatenate([P, R], axis=1).astype(np.float32)                    # (A, 128, NB)

    def split(x):
        hi = x.astype(BF)
        lo = (x - hi.astype(np.float32)).astype(BF)
        return hi, lo

    qwh, qwl = split(qw)
    prh, prl = split(pr)
    qwcat = np.stack([qwh, qwl], axis=1)                      # (128, 2, U)
    prcat = np.stack([prh, prl], axis=2)                      # (A, 128, 2, NB)

    # gph[a][i] = g[i - 127], g[k] = forces[a, 0, F-1-k]; bf16 hi/lo split
    g = forces[:, 0, ::-1].astype(np.float32)
    gp = np.zeros((A, GPH), np.float32)
    gp[:, 127:127 + F] = g
    gp_hi, gp_lo = split(gp)
    gcat = np.stack([gp_hi, gp_lo], axis=1)                   # (A, 2, GPH)
    # expanded Toeplitz: mtab[a, s, h, c] = gcat[a, h, s + c]
    win = np.lib.stride_tricks.sliding_window_view(gcat, D * 128, axis=2)
    # win: (A, 2, GPH - D*128 + 1, D*128); take s = 0..127 windows
    mtab = np.ascontiguousarray(win[:, :, :128].transpose(0, 2, 1, 3))
    return qwcat, prcat, mtab


def kernel(freq_linear, amp_value, alpha_params, beta_params, forces):
    freq_linear = np.asarray(freq_linear)
    amp_value = np.asarray(amp_value)
    alpha_params = np.asarray(alpha_params)
    beta_params = np.asarray(beta_params)
    forces = np.asarray(forces)

    qwcat, prcat, mtab = _host_tables(
        freq_linear, amp_value, alpha_params, beta_params, forces)

    nc = _build_program_raw()
    in_maps = []
    for c in range(NCORES):
        sl = slice(c * APC, (c + 1) * APC)
        in_maps.append({
            "qwcat": qwcat,
            "prcat": np.ascontiguousarray(prcat[sl]),
            "mtab": np.ascontiguousarray(mtab[sl]),
        })
    res = run_bass_kernel_spmd(nc, in_maps, core_ids=list(range(NCORES)))
    # device layout is [t0, T1]; linear time is 128*T1 + t0
    out = np.concatenate(
        [r["out"].transpose(0, 2, 1).reshape(APC, T) for r in res.results], axis=0)
    return out.astype(np.float32)


# revision 27
# speedup vs baseline: 1.0610x; 1.0610x over previous
"""DampedOscillator Trainium2 kernel.

Math: per mode m, damp/freq are constant in time, so with t = 128*B + u,
  exp(-damp*t/SR)          = E2[m,B] * E1[m,u]
  sin(2*pi*wd*t)           = S2[m,B]*C1[m,u] + C2[m,B]*S1[m,u]
so the amp-weighted mode sum
  signal[a, 128B+u] = sum_m P[a,m,B]*Q[m,u] + R[a,m,B]*W[m,u]
is one K=128 matmul per audio channel (tables host-built, O(M*(128+256))),
producing the signal directly in [u, B] layout.

The grouped long conv out[t] = sum_k g[k]*x[t-k] (g = time-flipped forces):
  out[128*T1 + t0] = sum_d sum_s M_d[s, t0] * x[s, T1-d],
  M_d[s, t0] = gph[s + 128*d + t0]
as 33 accumulating matmuls per channel with the Toeplitz slice M_d stationary
(reused across 256 moving columns) and the signal moving.

All matmuls run in bf16 hi/lo split form (hi*hi + hi*lo + lo*hi), keeping
~16 mantissa bits end to end at 1 PE column/cycle. Toeplitz tiles are built
on-device by overlapping (im2col) DMAs from host-split bf16 force vectors,
chunked along the tap axis and ordered so the PE can start as soon as the
first chunk lands while the rest stream in behind it.

Output lands as [t0, T1] per channel; the host transposes to linear time.

Sharding: data parallel over the 16 audio channels, 2 per NeuronCore.
"""
import ml_dtypes
import numpy as np

import bass_rust
import concourse.bass as bass
import concourse.mybir as mybir
from concourse import bacc
from concourse.bass_utils import run_bass_kernel_spmd

F32 = mybir.dt.float32
BF16 = mybir.dt.bfloat16
BF = ml_dtypes.bfloat16

SR = 16000.0
T = 32768
A = 16
MODES = 64
F = 4096
NB = 256          # number of 128-sample blocks
U = 128           # samples per block
D = 33            # kernel chunks (4096/128 + 1 boundary)
NCORES = 8
APC = A // NCORES  # audio channels per core
GPH = F + 256      # padded g vector length (127 zeros + 4096 + 129 zeros)
DGROUPS = (3, 4, 5, 7, 7, 7)  # d-axis DMA chunking for load/compute overlap

_program_cache = {}


def _toeplitz_src(src_t, a, d0, dn):
    src = src_t[a].copy()
    src.ap = bass_rust.VecI64Pair([[1, 128], [1, dn * 128]])
    src.offset = src.offset + d0 * 128
    return src


def _build_program_raw():
    """Hand-scheduled variant: no TileContext, manual semaphores, minimal
    DMA count (issue costs ~0.65us each), per-d weight-load pairing."""
    if "nc_raw" in _program_cache:
        return _program_cache["nc_raw"]

    nc = bacc.Bacc("TRN2", debug=False)

    qw_in = nc.dram_tensor("qwcat", [128, 2, U], BF16, kind="ExternalInput")
    pr_in = nc.dram_tensor("prcat", [APC, 128, 2, NB], BF16, kind="ExternalInput")
    g_in = nc.dram_tensor("mtab", [APC, 128, 2, D * 128], BF16,
                          kind="ExternalInput")
    out_t = nc.dram_tensor("out", [APC, U, NB], F32, kind="ExternalOutput")

    dg = list(DGROUPS)
    dg_off = [sum(dg[:i]) for i in range(len(dg))]
    import contextlib
    with contextlib.ExitStack() as ctx:
        sb = lambda shape, dt, name: ctx.enter_context(
            nc.sbuf_tensor(name, shape, dt))
        ps = lambda shape, name: ctx.enter_context(
            nc.psum_tensor(name, shape, F32))
        sem = lambda name: ctx.enter_context(nc.semaphore(name))

        qw = sb([128, 2, U], BF16, "qw_t")        # [k, hi/lo, u]
        pr = [sb([128, 2, NB], BF16, f"pr_t{a}") for a in range(APC)]
        mt = [[sb([128, 2, dn * 128], BF16, f"mt{a}g{g}")
               for g, dn in enumerate(dg)] for a in range(APC)]
        xh = [sb([128, 32 + NB], BF16, f"xh_t{a}") for a in range(APC)]
        xl = [sb([128, 32 + NB], BF16, f"xl_t{a}") for a in range(APC)]
        ob = [sb([128, NB], F32, f"ob_t{a}") for a in range(APC)]
        sig = [ps([128, NB], f"sig_t{a}") for a in range(APC)]
        acc = [ps([128, NB], f"acc_t{a}") for a in range(APC)]
        wup_sb = sb([128, 512], BF16, "wup_sb")
        wup_ps = ps([128, 512], "wup_ps")

        s_d0 = sem("s_d0")          # sync smalls: qwcat + prcat0
        s_d1 = sem("s_d1")          # gpsimd smalls: prcat1
        s_m0 = [sem(f"s_m0g{g}") for g in range(len(dg))]
        s_m1 = [sem(f"s_m1g{g}") for g in range(len(dg))]
        s_sig = sem("s_sig")
        s_split = sem("s_split")
        s_acc = sem("s_acc")
        s_ob = sem("s_ob")
        s_outd = sem("s_outd")

        def toe_src(a, d0, dn):
            return g_in[a, :, :, d0 * 128:(d0 + dn) * 128]

        with nc.Block() as block:
            @block.sync
            def _(sync):
                sync.dma_start(qw[:], qw_in[:]).then_inc(s_d0, 16)
                for g in range(1, len(dg), 2):
                    sync.dma_start(mt[0][g][:], toe_src(0, dg_off[g], dg[g])
                                   ).then_inc(s_m0[g], 16)
                for g in range(0, len(dg), 2):
                    sync.dma_start(mt[1][g][:], toe_src(1, dg_off[g], dg[g])
                                   ).then_inc(s_m1[g], 16)
                sync.wait_ge(s_ob, 1)
                sync.dma_start(out_t[0, 0:64], ob[0][0:64, :]).then_inc(s_outd, 16)
                sync.wait_ge(s_ob, 2)
                sync.dma_start(out_t[0, 64:128], ob[0][64:128, :]).then_inc(s_outd, 16)
                sync.wait_ge(s_ob, 3)
                sync.dma_start(out_t[1, 0:64], ob[1][0:64, :]).then_inc(s_outd, 16)
                sync.wait_ge(s_ob, 4)
                sync.dma_start(out_t[1, 64:128], ob[1][64:128, :]).then_inc(s_outd, 16)
                sync.wait_ge(s_outd, 64)

            @block.scalar
            def _(sc):
                sc.dma_start(pr[0][:], pr_in[0]).then_inc(s_d0, 16)
                for g in range(0, len(dg), 2):
                    sc.dma_start(mt[0][g][:], toe_src(0, dg_off[g], dg[g])
                                 ).then_inc(s_m0[g], 16)
                for g in range(1, len(dg), 2):
                    sc.dma_start(mt[1][g][:], toe_src(1, dg_off[g], dg[g])
                                 ).then_inc(s_m1[g], 16)

            @block.gpsimd
            def _(gp):
                gp.dma_start(pr[1][:], pr_in[1]).then_inc(s_d1, 16)

            @block.tensor
            def _(pe):
                # HAM warmup: keep the PE busy on scratch data while input
                # DMAs stream, so the real matmuls run at the warm clock
                for _ in range(13):
                    nc.tensor.matmul(wup_ps[:], wup_sb[:, 0:128], wup_sb[:],
                                     start=True, stop=True)
                pe.wait_ge(s_d0, 32)
                nc.tensor.matmul(sig[0][:], qw[:, 0], pr[0][:, 0], start=True, stop=False)
                nc.tensor.matmul(sig[0][:], qw[:, 0], pr[0][:, 1], start=False, stop=False)
                nc.tensor.matmul(sig[0][:], qw[:, 1], pr[0][:, 0], start=False,
                                 stop=True).then_inc(s_sig, 1)
                pe.wait_ge(s_d1, 16)
                nc.tensor.matmul(sig[1][:], qw[:, 0], pr[1][:, 0], start=True, stop=False)
                nc.tensor.matmul(sig[1][:], qw[:, 0], pr[1][:, 1], start=False, stop=False)
                nc.tensor.matmul(sig[1][:], qw[:, 1], pr[1][:, 0], start=False,
                                 stop=True).then_inc(s_sig, 1)

                def conv(a):
                    k = 0
                    n = 3 * D
                    inst = None
                    for g, dn in enumerate(dg):
                        pe.wait_ge(s_m0[g] if a == 0 else s_m1[g], 16)
                        for d in range(dg_off[g], dg_off[g] + dn):
                            hi = mt[a][g][:, 0, (d - dg_off[g]) * 128:
                                          (d - dg_off[g] + 1) * 128]
                            for rhs in (xh[a], xl[a]):
                                inst = nc.tensor.matmul(
                                    acc[a][:], hi, rhs[:, 32 - d:32 - d + NB],
                                    start=(k == 0), stop=(k == n - 1))
                                k += 1
                    for g, dn in enumerate(dg):
                        for d in range(dg_off[g], dg_off[g] + dn):
                            lo = mt[a][g][:, 1, (d - dg_off[g]) * 128:
                                          (d - dg_off[g] + 1) * 128]
                            inst = nc.tensor.matmul(
                                acc[a][:], lo, xh[a][:, 32 - d:32 - d + NB],
                                start=(k == 0), stop=(k == n - 1))
                            k += 1
                    inst.then_inc(s_acc, 1)

                pe.wait_ge(s_split, 2)
                conv(0)
                pe.wait_ge(s_split, 4)
                conv(1)

            @block.vector
            def _(v):
                for a in range(APC):
                    nc.vector.memset(xh[a][:, 0:32], 0.0)
                    nc.vector.memset(xl[a][:, 0:32], 0.0)
                for a in range(APC):
                    v.wait_ge(s_sig, a + 1)
                    nc.vector.tensor_copy(xh[a][:, 32:], sig[a][:]).then_inc(
                        s_split, 1)
                    nc.vector.tensor_sub(xl[a][:, 32:], sig[a][:],
                                         xh[a][:, 32:]).then_inc(s_split, 1)
                for a in range(APC):
                    v.wait_ge(s_acc, a + 1)
                    nc.vector.tensor_copy(ob[a][0:64, :],
                                          acc[a][0:64, :]).then_inc(s_ob, 1)
                    nc.vector.tensor_copy(ob[a][64:128, :],
                                          acc[a][64:128, :]).then_inc(s_ob, 1)

    nc.compile()
    _program_cache["nc_raw"] = nc
    return nc


def _host_tables(freq_linear, amp_value, alpha_params, beta_params, forces):
    """Derive per-mode constants and the synthesis tables in float64."""
    LOG10 = np.log(10.0)
    amp = 2.0 * (1.0 / (1.0 + np.exp(-amp_value.astype(np.float64)))) ** LOG10 + 1e-7

    def wsum(p, v):
        x = np.log1p(np.exp(p.astype(np.float64)))
        x = x / x.sum(-1, keepdims=True)
        return (v * x).sum(-1)

    alpha_list = np.exp(np.linspace(np.log(0.6), np.log(60.0), 64))
    beta_list = np.exp(np.linspace(np.log(2e-8), np.log(2e-6), 64))
    alpha = wsum(alpha_params, alpha_list)[0, :, 0]
    beta = wsum(beta_params, beta_list)[0, :, 0]
    fl = freq_linear.astype(np.float64)
    lbd = (2.0 * np.pi * fl) ** 2
    damp = 0.5 * (alpha + beta * lbd)
    freq = np.sqrt(lbd - damp ** 2) / (2.0 * np.pi)
    wd = freq / SR

    Bv = np.arange(NB, dtype=np.float64)
    uv = np.arange(U, dtype=np.float64)
    E2 = np.exp(-damp[:, None] * 128.0 * Bv[None, :] / SR)
    ph2 = (wd[:, None] * 128.0 * Bv[None, :]) % 1.0
    S2, C2 = np.sin(2 * np.pi * ph2), np.cos(2 * np.pi * ph2)
    E1 = np.exp(-damp[:, None] * (uv[None, :] + 1.0) / SR)
    ph1 = wd[:, None] * (uv[None, :] + 1.0)
    S1, C1 = np.sin(2 * np.pi * ph1), np.cos(2 * np.pi * ph1)

    Q = E1 * C1                                   # (M, U)
    W = E1 * S1
    P = amp[:, :, 0, None] * E2[None] * S2[None]  # (A, M, NB)
    R = amp[:, :, 0, None] * E2[None] * C2[None]

    # device lhsT: qw[k, s'] with s' = 127-u (reversed sample order), K = [Q; W]
    qw = np.concatenate([Q[:, ::-1], W[:, ::-1]], axis=0).astype(np.float32)  # (128, U)
    pr = np.concatenate([P, R], axis=1).astype(np.float32)                    # (A, 128, NB)

    def split(x):
        hi = x.astype(BF)
        lo = (x - hi.astype(np.float32)).astype(BF)
        return hi, lo

    qwh, qwl = split(qw)
    prh, prl = split(pr)
    qwcat = np.stack([qwh, qwl], axis=1)                      # (128, 2, U)
    prcat = np.stack([prh, prl], axis=2)                      # (A, 128, 2, NB)

    # gph[a][i] = g[i - 127], g[k] = forces[a, 0, F-1-k]; bf16 hi/lo split
    g = forces[:, 0, ::-1].astype(np.float32)
    gp = np.zeros((A, GPH), np.float32)
    gp[:, 127:127 + F] = g
    gp_hi, gp_lo = split(gp)
    gcat = np.stack([gp_hi, gp_lo], axis=1)                   # (A, 2, GPH)
    # expanded Toeplitz: mtab[a, s, h, c] = gcat[a, h, s + c]
    win = np.lib.stride_tricks.sliding_window_view(gcat, D * 128, axis=2)
    # win: (A, 2, GPH - D*128 + 1, D*128); take s = 0..127 windows
    mtab = np.ascontiguousarray(win[:, :, :128].transpose(0, 2, 1, 3))
    return qwcat, prcat, mtab


def kernel(freq_linear, amp_value, alpha_params, beta_params, forces):
    freq_linear = np.asarray(freq_linear)
    amp_value = np.asarray(amp_value)
    alpha_params = np.asarray(alpha_params)
    beta_params = np.asarray(beta_params)
    forces = np.asarray(forces)

    qwcat, prcat, mtab = _host_tables(
        freq_linear, amp_value, alpha_params, beta_params, forces)

    nc = _build_program_raw()
    in_maps = []
    for c in range(NCORES):
        sl = slice(c * APC, (c + 1) * APC)
        in_maps.append({
            "qwcat": qwcat,
            "prcat": np.ascontiguousarray(prcat[sl]),
            "mtab": np.ascontiguousarray(mtab[sl]),
        })
    res = run_bass_kernel_spmd(nc, in_maps, core_ids=list(range(NCORES)))
    # device layout is [t0, T1]; linear time is 128*T1 + t0
    out = np.concatenate(
        [r["out"].transpose(0, 2, 1).reshape(APC, T) for r in res.results], axis=0)
    return out.astype(np.float32)


# revision 31
# speedup vs baseline: 1.0697x; 1.0082x over previous
"""DampedOscillator Trainium2 kernel.

Math: per mode m, damp/freq are constant in time, so with t = 128*B + u,
  exp(-damp*t/SR)          = E2[m,B] * E1[m,u]
  sin(2*pi*wd*t)           = S2[m,B]*C1[m,u] + C2[m,B]*S1[m,u]
so the amp-weighted mode sum
  signal[a, 128B+u] = sum_m P[a,m,B]*Q[m,u] + R[a,m,B]*W[m,u]
is one K=128 matmul per audio channel (tables host-built, O(M*(128+256))),
producing the signal directly in [u, B] layout.

The grouped long conv out[t] = sum_k g[k]*x[t-k] (g = time-flipped forces):
  out[128*T1 + t0] = sum_d sum_s M_d[s, t0] * x[s, T1-d],
  M_d[s, t0] = gph[s + 128*d + t0]
as 33 accumulating matmuls per channel with the Toeplitz slice M_d stationary
(reused across 256 moving columns) and the signal moving.

All matmuls run in bf16 hi/lo split form (hi*hi + hi*lo + lo*hi), keeping
~16 mantissa bits end to end at 1 PE column/cycle. Toeplitz tiles are built
on-device by overlapping (im2col) DMAs from host-split bf16 force vectors,
chunked along the tap axis and ordered so the PE can start as soon as the
first chunk lands while the rest stream in behind it.

Output lands as [t0, T1] per channel; the host transposes to linear time.

Sharding: data parallel over the 16 audio channels, 2 per NeuronCore.
"""
import ml_dtypes
import numpy as np

import bass_rust
import concourse.bass as bass
import concourse.mybir as mybir
from concourse import bacc
from concourse.bass_utils import run_bass_kernel_spmd

F32 = mybir.dt.float32
BF16 = mybir.dt.bfloat16
BF = ml_dtypes.bfloat16

SR = 16000.0
T = 32768
A = 16
MODES = 64
F = 4096
NB = 256          # number of 128-sample blocks
U = 128           # samples per block
D = 33            # kernel chunks (4096/128 + 1 boundary)
NCORES = 8
APC = A // NCORES  # audio channels per core
GPH = F + 256      # padded g vector length (127 zeros + 4096 + 129 zeros)
DGROUPS = (3, 4, 5, 7, 7, 7)  # d-axis DMA chunking for load/compute overlap

_program_cache = {}


def _toeplitz_src(src_t, a, d0, dn):
    src = src_t[a].copy()
    src.ap = bass_rust.VecI64Pair([[1, 128], [1, dn * 128]])
    src.offset = src.offset + d0 * 128
    return src


def _build_program_raw():
    """Hand-scheduled variant: no TileContext, manual semaphores, minimal
    DMA count (issue costs ~0.65us each), per-d weight-load pairing."""
    if "nc_raw" in _program_cache:
        return _program_cache["nc_raw"]

    nc = bacc.Bacc("TRN2", debug=False)

    qw_in = nc.dram_tensor("qwcat", [128, 2, U], BF16, kind="ExternalInput")
    pr_in = nc.dram_tensor("prcat", [APC, 128, 2, NB], BF16, kind="ExternalInput")
    g_in = nc.dram_tensor("mtab", [APC, 128, 2, D * 128], BF16,
                          kind="ExternalInput")
    out_t = nc.dram_tensor("out", [APC, U, NB], F32, kind="ExternalOutput")

    dg = list(DGROUPS)
    dg_off = [sum(dg[:i]) for i in range(len(dg))]
    import contextlib
    with contextlib.ExitStack() as ctx:
        sb = lambda shape, dt, name: ctx.enter_context(
            nc.sbuf_tensor(name, shape, dt))
        ps = lambda shape, name: ctx.enter_context(
            nc.psum_tensor(name, shape, F32))
        sem = lambda name: ctx.enter_context(nc.semaphore(name))

        qw = sb([128, 2, U], BF16, "qw_t")        # [k, hi/lo, u]
        pr = [sb([128, 2, NB], BF16, f"pr_t{a}") for a in range(APC)]
        mt = [[sb([128, 2, dn * 128], BF16, f"mt{a}g{g}")
               for g, dn in enumerate(dg)] for a in range(APC)]
        xh = [sb([128, 32 + NB], BF16, f"xh_t{a}") for a in range(APC)]
        xl = [sb([128, 32 + NB], BF16, f"xl_t{a}") for a in range(APC)]
        ob = [sb([128, NB], F32, f"ob_t{a}") for a in range(APC)]
        sig = [ps([128, NB], f"sig_t{a}") for a in range(APC)]
        acc = [ps([128, NB], f"acc_t{a}") for a in range(APC)]
        wup_sb = sb([128, 512], BF16, "wup_sb")
        wup_ps = ps([128, 512], "wup_ps")

        s_d0 = sem("s_d0")          # sync smalls: qwcat + prcat0
        s_d1 = sem("s_d1")          # gpsimd smalls: prcat1
        s_m0 = [sem(f"s_m0g{g}") for g in range(len(dg))]
        s_m1 = [sem(f"s_m1g{g}") for g in range(len(dg))]
        s_sig = sem("s_sig")
        s_split = sem("s_split")
        s_acc = sem("s_acc")
        s_ob = sem("s_ob")
        s_outd = sem("s_outd")

        def toe_src(a, d0, dn):
            return g_in[a, :, :, d0 * 128:(d0 + dn) * 128]

        with nc.Block() as block:
            @block.sync
            def _(sync):
                sync.dma_start(qw[:], qw_in[:]).then_inc(s_d0, 16)
                for g in range(1, len(dg), 2):
                    sync.dma_start(mt[0][g][:], toe_src(0, dg_off[g], dg[g])
                                   ).then_inc(s_m0[g], 16)
                for g in range(0, len(dg), 2):
                    sync.dma_start(mt[1][g][:], toe_src(1, dg_off[g], dg[g])
                                   ).then_inc(s_m1[g], 16)
                sync.wait_ge(s_ob, 1)
                sync.dma_start(out_t[0, 0:64], ob[0][0:64, :]).then_inc(s_outd, 16)
                sync.wait_ge(s_ob, 2)
                sync.dma_start(out_t[0, 64:128], ob[0][64:128, :]).then_inc(s_outd, 16)
                sync.wait_ge(s_ob, 3)
                sync.dma_start(out_t[1, 0:64], ob[1][0:64, :]).then_inc(s_outd, 16)
                sync.wait_ge(s_ob, 4)
                sync.dma_start(out_t[1, 64:128], ob[1][64:128, :]).then_inc(s_outd, 16)
                sync.wait_ge(s_outd, 64)

            @block.scalar
            def _(sc):
                sc.dma_start(pr[0][:], pr_in[0]).then_inc(s_d0, 16)
                for g in range(0, len(dg), 2):
                    sc.dma_start(mt[0][g][:], toe_src(0, dg_off[g], dg[g])
                                 ).then_inc(s_m0[g], 16)
                for g in range(1, len(dg), 2):
                    sc.dma_start(mt[1][g][:], toe_src(1, dg_off[g], dg[g])
                                 ).then_inc(s_m1[g], 16)

            @block.gpsimd
            def _(gp):
                gp.dma_start(pr[1][:], pr_in[1]).then_inc(s_d1, 16)

            @block.tensor
            def _(pe):
                # HAM warmup: keep the PE busy on scratch data while input
                # DMAs stream, so the real matmuls run at the warm clock
                for _ in range(13):
                    nc.tensor.matmul(wup_ps[:], wup_sb[:, 0:128], wup_sb[:],
                                     start=True, stop=True)
                pe.wait_ge(s_d0, 32)
                nc.tensor.matmul(sig[0][:], qw[:, 0], pr[0][:, 0], start=True, stop=False)
                nc.tensor.matmul(sig[0][:], qw[:, 0], pr[0][:, 1], start=False, stop=False)
                nc.tensor.matmul(sig[0][:], qw[:, 1], pr[0][:, 0], start=False,
                                 stop=True).then_inc(s_sig, 1)
                pe.wait_ge(s_d1, 16)
                nc.tensor.matmul(sig[1][:], qw[:, 0], pr[1][:, 0], start=True, stop=False)
                nc.tensor.matmul(sig[1][:], qw[:, 0], pr[1][:, 1], start=False, stop=False)
                nc.tensor.matmul(sig[1][:], qw[:, 1], pr[1][:, 0], start=False,
                                 stop=True).then_inc(s_sig, 1)

                def conv(a):
                    k = 0
                    n = 3 * D
                    inst = None
                    for g, dn in enumerate(dg):
                        pe.wait_ge(s_m0[g] if a == 0 else s_m1[g], 16)
                        for d in range(dg_off[g], dg_off[g] + dn):
                            hi = mt[a][g][:, 0, (d - dg_off[g]) * 128:
                                          (d - dg_off[g] + 1) * 128]
                            for rhs in (xh[a], xl[a]):
                                inst = nc.tensor.matmul(
                                    acc[a][:], hi, rhs[:, 32 - d:32 - d + NB],
                                    start=(k == 0), stop=(k == n - 1))
                                k += 1
                    for g, dn in enumerate(dg):
                        for d in range(dg_off[g], dg_off[g] + dn):
                            lo = mt[a][g][:, 1, (d - dg_off[g]) * 128:
                                          (d - dg_off[g] + 1) * 128]
                            inst = nc.tensor.matmul(
                                acc[a][:], lo, xh[a][:, 32 - d:32 - d + NB],
                                start=(k == 0), stop=(k == n - 1))
                            k += 1
                    inst.then_inc(s_acc, 1)

                pe.wait_ge(s_split, 2)
                conv(0)
                pe.wait_ge(s_split, 4)
                conv(1)

            @block.vector
            def _(v):
                for a in range(APC):
                    nc.vector.memset(xh[a][:, 0:32], 0.0)
                    nc.vector.memset(xl[a][:, 0:32], 0.0)
                for a in range(APC):
                    v.wait_ge(s_sig, a + 1)
                    nc.vector.tensor_copy(xh[a][:, 32:], sig[a][:]).then_inc(
                        s_split, 1)
                    nc.vector.tensor_sub(xl[a][:, 32:], sig[a][:],
                                         xh[a][:, 32:]).then_inc(s_split, 1)
                for a in range(APC):
                    v.wait_ge(s_acc, a + 1)
                    nc.vector.tensor_copy(ob[a][0:64, :],
                                          acc[a][0:64, :]).then_inc(s_ob, 1)
                    nc.vector.tensor_copy(ob[a][64:128, :],
                                          acc[a][64:128, :]).then_inc(s_ob, 1)

    nc.compile()
    _program_cache["nc_raw"] = nc
    return nc


def _host_tables(freq_linear, amp_value, alpha_params, beta_params, forces):
    """Derive per-mode constants and the synthesis tables in float64."""
    LOG10 = np.log(10.0)
    amp = 2.0 * (1.0 / (1.0 + np.exp(-amp_value.astype(np.float64)))) ** LOG10 + 1e-7

    def wsum(p, v):
        x = np.log1p(np.exp(p.astype(np.float64)))
        x = x / x.sum(-1, keepdims=True)
        return (v * x).sum(-1)

    alpha_list = np.exp(np.linspace(np.log(0.6), np.log(60.0), 64))
    beta_list = np.exp(np.linspace(np.log(2e-8), np.log(2e-6), 64))
    alpha = wsum(alpha_params, alpha_list)[0, :, 0]
    beta = wsum(beta_params, beta_list)[0, :, 0]
    fl = freq_linear.astype(np.float64)
    lbd = (2.0 * np.pi * fl) ** 2
    damp = 0.5 * (alpha + beta * lbd)
    freq = np.sqrt(lbd - damp ** 2) / (2.0 * np.pi)
    wd = freq / SR

    Bv = np.arange(NB, dtype=np.float64)
    uv = np.arange(U, dtype=np.float64)
    E2 = np.exp(-damp[:, None] * 128.0 * Bv[None, :] / SR)
    ph2 = (wd[:, None] * 128.0 * Bv[None, :]) % 1.0
    S2, C2 = np.sin(2 * np.pi * ph2), np.cos(2 * np.pi * ph2)
    E1 = np.exp(-damp[:, None] * (uv[None, :] + 1.0) / SR)
    ph1 = wd[:, None] * (uv[None, :] + 1.0)
    S1, C1 = np.sin(2 * np.pi * ph1), np.cos(2 * np.pi * ph1)

    Q = E1 * C1                                   # (M, U)
    W = E1 * S1
    P = amp[:, :, 0, None] * E2[None] * S2[None]  # (A, M, NB)
    R = amp[:, :, 0, None] * E2[None] * C2[None]

    # device lhsT: qw[k, s'] with s' = 127-u (reversed sample order), K = [Q; W]
    qw = np.concatenate([Q[:, ::-1], W[:, ::-1]], axis=0).astype(np.float32)  # (128, U)
    pr = np.concatenate([P, R], axis=1).astype(np.float32)                    # (A, 128, NB)

    def split(x):
        hi = x.astype(BF)
        lo = (x - hi.astype(np.float32)).astype(BF)
        return hi, lo

    qwh, qwl = split(qw)
    prh, prl = split(pr)
    qwcat = np.stack([qwh, qwl], axis=1)                      # (128, 2, U)
    prcat = np.stack([prh, prl], axis=2)                      # (A, 128, 2, NB)

    # gph[a][i] = g[i - 127], g[k] = forces[a, 0, F-1-k]; bf16 hi/lo split
    g = forces[:, 0, ::-1].astype(np.float32)
    gp = np.zeros((A, GPH), np.float32)
    gp[:, 127:127 + F] = g
    gp_hi, gp_lo = split(gp)
    gcat = np.stack([gp_hi, gp_lo], axis=1)                   # (A, 2, GPH)
    # expanded Toeplitz: mtab[a, s, h, c] = gcat[a, h, s + c]
    win = np.lib.stride_tricks.sliding_window_view(gcat, D * 128, axis=2)
    # win: (A, 2, GPH - D*128 + 1, D*128); take s = 0..127 windows
    mtab = np.ascontiguousarray(win[:, :, :128].transpose(0, 2, 1, 3))
    return qwcat, prcat, mtab


def kernel(freq_linear, amp_value, alpha_params, beta_params, forces):
    freq_linear = np.asarray(freq_linear)
    amp_value = np.asarray(amp_value)
    alpha_params = np.asarray(alpha_params)
    beta_params = np.asarray(beta_params)
    forces = np.asarray(forces)

    qwcat, prcat, mtab = _host_tables(
        freq_linear, amp_value, alpha_params, beta_params, forces)

    nc = _build_program_raw()
    in_maps = []
    for c in range(NCORES):
        sl = slice(c * APC, (c + 1) * APC)
        in_maps.append({
            "qwcat": qwcat,
            "prcat": np.ascontiguousarray(prcat[sl]),
            "mtab": np.ascontiguousarray(mtab[sl]),
        })
    res = run_bass_kernel_spmd(nc, in_maps, core_ids=list(range(NCORES)))
    # device layout is [t0, T1]; linear time is 128*T1 + t0
    out = np.concatenate(
        [r["out"].transpose(0, 2, 1).reshape(APC, T) for r in res.results], axis=0)
    return out.astype(np.float32)
